# revision 14
# baseline (speedup 1.0000x reference)
"""Cost-volume layer (17-shift cross pattern, R=4) for Trainium2, 8 NeuronCores.

out[b,s,h,w] = sum_c src[b,c,h,w] * tgt[b,c,h+dh_s,w+dw_s]   (tgt zero-padded)

Strategy
--------
Shard: 8 cores = batch(4) x H-halves(2). Per core: src [128, 48*160],
tgt (padded, with halo) [128, 56*168]. C=128 lives in the SBUF partition
dim and is contracted on the TensorEngine via *banded correlations*:

- vertical shifts  (dh=-4..4, dw=0): per column w, matmul
    src[:, :, w]^T @ tgt[:, :, w+4]  ->  [48, 56] band matrix
- horizontal shifts (dh=0, dw=-4..4): per row h, per 32-col chunk i, matmul
    src[:, h, 32i:32i+32]^T @ tgt[:, h+4, 32i:32i+40] -> [32, 40] band matrix

Useful entries are the 9 diagonals of each band; bands are packed into PSUM
banks (several chunks per bank at 32-aligned partition bases), staged to SBUF
(DVE/ACT copies), DMA'd to HBM, and the diagonals are gathered host-side
(pure indexing - no host arithmetic).

Input loads are sliced by row-groups so horizontal banks start while the
tail of the inputs is still loading; band writes go out on the ACT HWDGE
ring so they do not serialize against input loads on the sync ring.
"""

import numpy as np
from contextlib import ExitStack

import concourse.bacc as bacc
import concourse.tile as tile
from concourse import mybir
from concourse import bass_utils

R = 4
B, C, H, W = 4, 128, 96, 160
NCORES = 8
HSH = H // 2            # 48 output rows per shard
HT = HSH + 2 * R        # 56 tgt rows (with halo)
WP = W + 2 * R          # 168 padded width
F32 = mybir.dt.float32

# compute dtype for the matmul inputs. float16 runs the PE at full rate
# (1 cyc/row vs fp32's 4) and halves input DMA, at ~3e-4 relative error
# (randn inputs are far from fp16 overflow). "float32" is the exact fallback.
COMPUTE_DT = "float16"

# vertical pass: H-halves of 24 rows so the top half can compute before the
# bottom rows finish loading. Per (w, half): matmul M=24 (src rows), N=32
# (tgt rows with halo). Chunks pack 16 per 32-aligned partition base x 4
# bases = 64 chunks (w's) per PSUM bank; 3 banks per half (64+64+32 w).
VM = 24
VN = VM + 2 * R          # 32
VSLOT = 16               # chunks per partition-base per bank
VBASES = (0, 32, 64, 96)
VPERBANK = VSLOT * len(VBASES)   # 64 w per bank (per half)
NVBANK = (W + VPERBANK - 1) // VPERBANK  # 3 banks per half

# horizontal pass: chunks of 32 src cols, window N=40; pack 4 groups
# (bases 0,32,64,96) x 12 slots per bank -> 48 chunks per bank
MH = 32
NH = MH + 2 * R         # 40
NCH = W // MH           # 5 chunks per row
NQ = HSH * NCH          # 240 chunks total
HSLOT = 12
HBASES = (0, 32, 64, 96)
HPERBANK = HSLOT * len(HBASES)          # 48
NHBANK = (NQ + HPERBANK - 1) // HPERBANK  # 5

SHIFTS = [(0, 0)]
for i in range(1, R + 1):
    SHIFTS.extend([(-i, 0), (i, 0), (0, -i), (0, i)])

# input load row-slices (3 pieces each, larger runs for DMA efficiency):
#   horiz banks 0-2 + top-half verticals unlock after piece 2;
#   everything else after piece 3.
TGT_CUTS = [0, 20, 38, 56]
SRC_CUTS = [0, 16, 32, 48]


def build_nc():
    cdt = getattr(mybir.dt, COMPUTE_DT)
    bdt = mybir.dt.float16 if COMPUTE_DT != "float32" else F32
    nc = bacc.Bacc("TRN2", target_bir_lowering=False)
    src = nc.dram_tensor("src", [C, HSH * W], cdt, kind="ExternalInput")
    tgt = nc.dram_tensor("tgt", [C, HT * WP], cdt, kind="ExternalInput")
    # band layouts are DMA-run-friendly: one DMA per (half, base-group g)
    # writes multi-KB contiguous runs per partition
    vband = nc.dram_tensor("vband", [2, len(VBASES), VM, NVBANK, VSLOT, VN],
                           bdt, kind="ExternalOutput")
    hband = nc.dram_tensor("hband", [MH, len(HBASES), NHBANK * HSLOT, NH], bdt,
                           kind="ExternalOutput")

    with ExitStack() as ctx:
        tc = ctx.enter_context(tile.TileContext(nc))
        ins = ctx.enter_context(tc.tile_pool(name="ins", bufs=1))
        psum = ctx.enter_context(tc.tile_pool(name="psum", bufs=4, space="PSUM"))
        stage = ctx.enter_context(tc.tile_pool(name="stage", bufs=6))

        src_sb = ins.tile([C, HSH * W], cdt)
        tgt_sb = ins.tile([C, HT * WP], cdt)

        def load_piece(i):
            t0, t1 = TGT_CUTS[i] * WP, TGT_CUTS[i + 1] * WP
            s0, s1 = SRC_CUTS[i] * W, SRC_CUTS[i + 1] * W
            nc.sync.dma_start(out=tgt_sb[:, t0:t1], in_=tgt[:][:, t0:t1])
            nc.sync.dma_start(out=src_sb[:, s0:s1], in_=src[:][:, s0:s1])

        src3 = src_sb.rearrange("c (h w) -> c h w", w=W)
        tgt3 = tgt_sb.rearrange("c (h w) -> c h w", w=WP)

        copy_flip = [0]

        def stage_copy(dst, src_ap):
            # alternate PSUM->SBUF copies between DVE and ACT
            if copy_flip[0] % 2 == 0:
                nc.vector.tensor_copy(out=dst, in_=src_ap)
            else:
                nc.scalar.copy(out=dst, in_=src_ap)
            copy_flip[0] += 1

        def horiz_bank(bank, st, k):
            q0 = bank * HPERBANK
            pt = psum.tile([128, HSLOT * NH], F32, tag="hp")
            for g, base in enumerate(HBASES):
                for j in range(HSLOT):
                    q = q0 + g * HSLOT + j
                    h, i = divmod(q, NCH)
                    w0 = i * MH
                    nc.tensor.matmul(
                        out=pt[base:base + MH, j * NH:(j + 1) * NH],
                        lhsT=src3[:, h, w0:w0 + MH],
                        rhs=tgt3[:, h + R, w0:w0 + NH],
                        start=True, stop=True,
                        tile_position=(0, base),
                    )
            seg = HSLOT * NH
            stage_copy(st[:, k * seg:(k + 1) * seg], pt)

        def horiz_flush(st, grp):
            nb, b0 = len(grp), grp[0]
            seg = HSLOT * NH
            for g, base in enumerate(HBASES):
                nc.scalar.dma_start(
                    out=hband[:][:, g, b0 * HSLOT:(b0 + nb) * HSLOT, :],
                    in_=st[base:base + MH, :nb * seg],
                )

        def v_gs(bank):
            # number of used partition-base groups in this bank (per half)
            nw = min(VPERBANK, W - bank * VPERBANK)
            return (nw + VSLOT - 1) // VSLOT

        def vert_bank(half, bank, st, k):
            # chunk L = g*16+j covers w = bank*64 + L; src rows half*24+.,
            # tgt window rows half*24 .. half*24+32
            w0 = bank * VPERBANK
            hb = half * VM
            pt = psum.tile([120, VSLOT * VN], F32, tag="vp")
            for g in range(v_gs(bank)):
                base = VBASES[g]
                for j in range(VSLOT):
                    w = w0 + g * VSLOT + j
                    nc.tensor.matmul(
                        out=pt[base:base + VM, j * VN:(j + 1) * VN],
                        lhsT=src3[:, hb:hb + VM, w],
                        rhs=tgt3[:, hb:hb + VN, w + R],
                        start=True, stop=True,
                        tile_position=(0, base),
                    )
            seg = VSLOT * VN
            for g in range(v_gs(bank)):
                base = VBASES[g]
                stage_copy(
                    st[base:base + VM, k * seg:(k + 1) * seg],
                    pt[base:base + VM, :],
                )

        def vert_flush(half, st, grp):
            seg = VSLOT * VN
            for g, base in enumerate(VBASES):
                nb = sum(1 for b in grp if v_gs(b) > g)
                if nb > 0:
                    nc.scalar.dma_start(
                        out=vband[:][half, g, :, grp[0]:grp[0] + nb, :, :],
                        in_=st[base:base + VM, :nb * seg],
                    )

        HGRP = [[0, 1, 2], [3, 4]]
        VGRP = [0, 1, 2]      # vertical banks per half
        hseg = HSLOT * NH

        # pipeline: loads piecewise; horiz banks 0-2 and top-half verticals
        # unlock after piece 2; the rest needs piece 3.
        load_piece(0)
        load_piece(1)
        hst = stage.tile([128, 3 * hseg], bdt, tag="hs")
        horiz_bank(0, hst, 0)
        horiz_bank(1, hst, 1)
        load_piece(2)
        vst_t = stage.tile([120, 3 * VSLOT * VN], bdt, tag="vs")
        vert_bank(0, 0, vst_t, 0)
        horiz_bank(2, hst, 2)
        horiz_flush(hst, HGRP[0])
        vert_bank(0, 1, vst_t, 1)
        vert_bank(0, 2, vst_t, 2)
        vert_flush(0, vst_t, VGRP)
        hst2 = stage.tile([128, 3 * hseg], bdt, tag="hs")
        horiz_bank(3, hst2, 0)
        horiz_bank(4, hst2, 1)
        horiz_flush(hst2, HGRP[1])
        vst_b = stage.tile([120, 3 * VSLOT * VN], bdt, tag="vs")
        for k, bank in enumerate(VGRP):
            vert_bank(1, bank, vst_b, k)
        vert_flush(1, vst_b, VGRP)

    nc.compile()
    return nc


_NC_CACHE = []


def _get_nc():
    if not _NC_CACHE:
        _NC_CACHE.append(build_nc())
    return _NC_CACHE[0]


def shard_inputs(src, tgt):
    if COMPUTE_DT == "float32":
        np_cdt = np.float32
    elif COMPUTE_DT == "float16":
        np_cdt = np.float16
    else:
        import ml_dtypes
        np_cdt = np.dtype(ml_dtypes.bfloat16)
    src = np.asarray(src, dtype=np.float32)
    tgt = np.asarray(tgt, dtype=np.float32)
    tp = np.pad(tgt, ((0, 0), (0, 0), (R, R), (R, R)))
    in_maps = []
    for core in range(NCORES):
        b, hh = divmod(core, 2)
        h0 = hh * HSH
        s = np.ascontiguousarray(src[b, :, h0:h0 + HSH, :]).reshape(C, HSH * W)
        t = np.ascontiguousarray(tp[b, :, h0:h0 + HT, :]).reshape(C, HT * WP)
        in_maps.append({"src": s.astype(np_cdt), "tgt": t.astype(np_cdt)})
    return in_maps


def extract_output(results):
    """results: list of 8 dicts with
    'vband' [2, 48, NVBANK, 9, 56], 'hband' [32, 4, NHBANK*12, 40]."""
    out = np.zeros((B, len(SHIFTS), H, W), np.float32)
    hidx = np.arange(HSH)
    midx = np.arange(MH)
    widx = np.arange(W)
    iidx = np.arange(NCH)
    for core in range(NCORES):
        b, hh = divmod(core, 2)
        h0 = hh * HSH
        # [half,g,m,bank,j,n] -> [half, m, w=bank*64+g*16+j, n]
        vb = np.asarray(results[core]["vband"]).astype(np.float32)
        vb = vb.transpose(0, 2, 3, 1, 4, 5).reshape(
            2, VM, NVBANK * len(VBASES) * VSLOT, VN)[:, :, :W, :]
        # [m,g,bank*12+j,n] -> [m, q=bank*48+g*12+j, n] -> [m,h,i,n]
        hb = np.asarray(results[core]["hband"]).astype(np.float32)
        hb = hb.reshape(MH, len(HBASES), NHBANK, HSLOT, NH)
        hb = hb.transpose(0, 2, 1, 3, 4).reshape(MH, NQ, NH)
        hb = hb.reshape(MH, HSH, NCH, NH)
        midx24 = np.arange(VM)
        for s, (dh, dw) in enumerate(SHIFTS):
            if dw == 0:
                for half in (0, 1):
                    out[b, s, h0 + half * VM:h0 + (half + 1) * VM, :] = vb[
                        half, midx24[:, None], widx[None, :],
                        (midx24 + dh + R)[:, None]
                    ]
            else:
                v = hb[
                    midx[:, None, None],
                    hidx[None, :, None],
                    iidx[None, None, :],
                    (midx + dw + R)[:, None, None],
                ]  # [m, h, i]
                out[b, s, h0:h0 + HSH, :] = v.transpose(1, 2, 0).reshape(HSH, W)
    return out


def kernel(src, tgt, **run_kwargs):
    nc = _get_nc()
    in_maps = shard_inputs(src, tgt)
    res = bass_utils.run_bass_kernel_spmd(
        nc, in_maps, core_ids=list(range(NCORES)), **run_kwargs
    )
    out = extract_output(res.results)
    kernel.last_result = res
    return out


# revision 20
# speedup vs baseline: 1.1281x; 1.1281x over previous
"""Cost-volume layer (17-shift cross pattern, R=4) for Trainium2, 8 NeuronCores.

out[b,s,h,w] = sum_c src[b,c,h,w] * tgt[b,c,h+dh_s,w+dw_s]   (tgt zero-padded)

Strategy
--------
Shard: 8 cores = batch(4) x H-halves(2). Per core: src [128, 48*160],
tgt (padded, with halo) [128, 56*168]. C=128 lives in the SBUF partition
dim and is contracted on the TensorEngine via *banded correlations*:

- vertical shifts  (dh=-4..4, dw=0): per column w, matmul
    src[:, :, w]^T @ tgt[:, :, w+4]  ->  [48, 56] band matrix
- horizontal shifts (dh=0, dw=-4..4): per row h, per 32-col chunk i, matmul
    src[:, h, 32i:32i+32]^T @ tgt[:, h+4, 32i:32i+40] -> [32, 40] band matrix

Useful entries are the 9 diagonals of each band; bands are packed into PSUM
banks (several chunks per bank at 32-aligned partition bases), staged to SBUF
(DVE/ACT copies), DMA'd to HBM, and the diagonals are gathered host-side
(pure indexing - no host arithmetic).

Input loads are sliced by row-groups so horizontal banks start while the
tail of the inputs is still loading; band writes go out on the ACT HWDGE
ring so they do not serialize against input loads on the sync ring.
"""

import numpy as np
from contextlib import ExitStack

import concourse.bacc as bacc
import concourse.tile as tile
from concourse import mybir
from concourse import bass_utils

R = 4
B, C, H, W = 4, 128, 96, 160
NCORES = 8
HSH = H // 2            # 48 output rows per shard
HT = HSH + 2 * R        # 56 tgt rows (with halo)
WP = W + 2 * R          # 168 padded width
F32 = mybir.dt.float32

# compute dtype for the matmul inputs. float16 runs the PE at full rate
# (1 cyc/row vs fp32's 4) and halves input DMA, at ~3e-4 relative error
# (randn inputs are far from fp16 overflow). "float32" is the exact fallback.
COMPUTE_DT = "float16"

# vertical pass: per-w matmul M=48, N=56; pack 2 groups (part base 0, 64)
# x 9 w-slots per PSUM bank -> 18 w per bank
VSLOT = 9
VBASES = (0, 64)
VPERBANK = VSLOT * len(VBASES)          # 18
NVBANK = (W + VPERBANK - 1) // VPERBANK  # 9

# horizontal pass: chunks of 32 src cols, window N=40; pack 4 groups
# (bases 0,32,64,96) x 12 slots per bank -> 48 chunks per bank
MH = 32
NH = MH + 2 * R         # 40
NCH = W // MH           # 5 chunks per row
NQ = HSH * NCH          # 240 chunks total
HSLOT = 12
HBASES = (0, 32, 64, 96)
HPERBANK = HSLOT * len(HBASES)          # 48
NHBANK = (NQ + HPERBANK - 1) // HPERBANK  # 5

SHIFTS = [(0, 0)]
for i in range(1, R + 1):
    SHIFTS.extend([(-i, 0), (i, 0), (0, -i), (0, i)])

# input load row-slices: tgt rows [0,16,32,44,56), src rows [0,12,24,36,48)
TGT_CUTS = [0, 16, 32, 44, 56]
SRC_CUTS = [0, 12, 24, 36, 48]
# horizontal bank b covers h in [b*48/5, ...]; bank ready after these pieces:
#   bank0: h<=9  -> tgt rows <=13 (piece 1), src rows <=9  (piece 1)
#   bank1: h<=19 -> tgt <=23 (piece 2), src <=19 (piece 2)
#   bank2: h<=28 -> tgt <=32 (piece 3), src <=28 (piece 3)
#   bank3: h<=38 -> tgt <=42 (piece 3), src <=38 (piece 4)
#   bank4: h<=47 -> all


def build_nc():
    cdt = getattr(mybir.dt, COMPUTE_DT)
    bdt = mybir.dt.float16 if COMPUTE_DT != "float32" else F32
    nc = bacc.Bacc("TRN2", target_bir_lowering=False)
    src = nc.dram_tensor("src", [C, HSH * W], cdt, kind="ExternalInput")
    tgt = nc.dram_tensor("tgt", [C, HT * WP], cdt, kind="ExternalInput")
    # band layouts are DMA-run-friendly: vband is flushed with ONE dma per
    # 3-bank group spanning all 112 stage partitions (hole rows memset by
    # the otherwise-idle GPSIMD), so every DMA port is active with 3KB runs
    vband = nc.dram_tensor("vband", [3, 112, 3, VSLOT * HT], bdt,
                           kind="ExternalOutput")
    hband = nc.dram_tensor("hband", [MH, len(HBASES), NHBANK * HSLOT, NH], bdt,
                           kind="ExternalOutput")

    with ExitStack() as ctx:
        tc = ctx.enter_context(tile.TileContext(nc))
        ins = ctx.enter_context(tc.tile_pool(name="ins", bufs=1))
        psum = ctx.enter_context(tc.tile_pool(name="psum", bufs=4, space="PSUM"))
        stage = ctx.enter_context(tc.tile_pool(name="stage", bufs=6))

        src_sb = ins.tile([C, HSH * W], cdt)
        tgt_sb = ins.tile([C, HT * WP], cdt)

        def load_piece(i):
            t0, t1 = TGT_CUTS[i] * WP, TGT_CUTS[i + 1] * WP
            s0, s1 = SRC_CUTS[i] * W, SRC_CUTS[i + 1] * W
            nc.sync.dma_start(out=tgt_sb[:, t0:t1], in_=tgt[:][:, t0:t1])
            nc.sync.dma_start(out=src_sb[:, s0:s1], in_=src[:][:, s0:s1])

        src3 = src_sb.rearrange("c (h w) -> c h w", w=W)
        tgt3 = tgt_sb.rearrange("c (h w) -> c h w", w=WP)

        copy_flip = [0]

        def stage_copy(dst, src_ap):
            # alternate PSUM->SBUF copies between DVE and ACT
            if copy_flip[0] % 2 == 0:
                nc.vector.tensor_copy(out=dst, in_=src_ap)
            else:
                nc.scalar.copy(out=dst, in_=src_ap)
            copy_flip[0] += 1

        def horiz_bank(bank, st, k):
            q0 = bank * HPERBANK
            pt = psum.tile([128, HSLOT * NH], F32, tag="hp")
            for g, base in enumerate(HBASES):
                for j in range(HSLOT):
                    q = q0 + g * HSLOT + j
                    h, i = divmod(q, NCH)
                    w0 = i * MH
                    nc.tensor.matmul(
                        out=pt[base:base + MH, j * NH:(j + 1) * NH],
                        lhsT=src3[:, h, w0:w0 + MH],
                        rhs=tgt3[:, h + R, w0:w0 + NH],
                        start=True, stop=True,
                        tile_position=(0, base),
                    )
            seg = HSLOT * NH
            stage_copy(st[:, k * seg:(k + 1) * seg], pt)

        def horiz_flush(st, grp):
            nb, b0 = len(grp), grp[0]
            seg = HSLOT * NH
            for g, base in enumerate(HBASES):
                nc.scalar.dma_start(
                    out=hband[:][:, g, b0 * HSLOT:(b0 + nb) * HSLOT, :],
                    in_=st[base:base + MH, :nb * seg],
                )

        def v_ng(bank, g):
            return min(VSLOT, max(0, min(VPERBANK, W - bank * VPERBANK) - g * VSLOT))

        def vert_bank(bank, st, k):
            w0 = bank * VPERBANK
            pt = psum.tile([112, VSLOT * HT], F32, tag="vp")
            for g, base in enumerate(VBASES):
                for j in range(v_ng(bank, g)):
                    w = w0 + g * VSLOT + j
                    nc.tensor.matmul(
                        out=pt[base:base + HSH, j * HT:(j + 1) * HT],
                        lhsT=src3[:, :, w],
                        rhs=tgt3[:, 0:HT, w + R],
                        start=True, stop=True,
                        tile_position=(0, base),
                    )
            seg = VSLOT * HT
            for g, base in enumerate(VBASES):
                ng = v_ng(bank, g)
                if ng > 0:
                    stage_copy(
                        st[base:base + HSH, k * seg:k * seg + ng * HT],
                        pt[base:base + HSH, :ng * HT],
                    )

        def vert_flush(st, gi):
            nc.scalar.dma_start(out=vband[:][gi], in_=st[0:112, :])

        HGRP = [[0, 1, 2], [3, 4]]
        VGRP = [[0, 1, 2], [3, 4, 5], [6, 7, 8]]
        hseg, vseg = HSLOT * NH, VSLOT * HT

        # pipeline: issue loads piecewise; horizontal banks unlock as the
        # rows they need land; vertical banks need all pieces.
        load_piece(0)
        load_piece(1)
        hst = stage.tile([128, 3 * hseg], bdt, tag="hs")
        horiz_bank(0, hst, 0)
        load_piece(2)
        horiz_bank(1, hst, 1)
        load_piece(3)
        horiz_bank(2, hst, 2)
        horiz_flush(hst, HGRP[0])
        hst2 = stage.tile([128, 3 * hseg], bdt, tag="hs")
        horiz_bank(3, hst2, 0)
        horiz_bank(4, hst2, 1)
        horiz_flush(hst2, HGRP[1])
        for gi, grp in enumerate(VGRP):
            vst = stage.tile([112, 3 * vseg], bdt, tag="vs")
            # zero hole partitions (48:64) + group-B rows so one whole-tile
            # DMA per group is fully initialized; copies overwrite the rest
            nc.gpsimd.memset(vst, 0.0)
            for k, bank in enumerate(grp):
                vert_bank(bank, vst, k)
            vert_flush(vst, gi)

    nc.compile()
    return nc


_NC_CACHE = []


def _get_nc():
    if not _NC_CACHE:
        _NC_CACHE.append(build_nc())
    return _NC_CACHE[0]


def shard_inputs(src, tgt):
    if COMPUTE_DT == "float32":
        np_cdt = np.float32
    elif COMPUTE_DT == "float16":
        np_cdt = np.float16
    else:
        import ml_dtypes
        np_cdt = np.dtype(ml_dtypes.bfloat16)
    src = np.asarray(src, dtype=np.float32)
    tgt = np.asarray(tgt, dtype=np.float32)
    tp = np.pad(tgt, ((0, 0), (0, 0), (R, R), (R, R)))
    in_maps = []
    for core in range(NCORES):
        b, hh = divmod(core, 2)
        h0 = hh * HSH
        s = np.ascontiguousarray(src[b, :, h0:h0 + HSH, :]).reshape(C, HSH * W)
        t = np.ascontiguousarray(tp[b, :, h0:h0 + HT, :]).reshape(C, HT * WP)
        in_maps.append({"src": s.astype(np_cdt), "tgt": t.astype(np_cdt)})
    return in_maps


def extract_output(results):
    """results: list of 8 dicts with
    'vband' [2, 48, NVBANK, 9, 56], 'hband' [32, 4, NHBANK*12, 40]."""
    out = np.zeros((B, len(SHIFTS), H, W), np.float32)
    hidx = np.arange(HSH)
    midx = np.arange(MH)
    widx = np.arange(W)
    iidx = np.arange(NCH)
    for core in range(NCORES):
        b, hh = divmod(core, 2)
        h0 = hh * HSH
        # [grp, p, bank, j*56+h'] -> [h, w=grp*54+bank*18+g*9+j, h']
        # where p = 64*g + h (partition groups at 0 and 64, holes 48:64)
        vb = np.asarray(results[core]["vband"]).astype(np.float32)
        vb = vb.reshape(3, 112, 3, VSLOT, HT)
        vb = np.stack([vb[:, 0:HSH], vb[:, 64:64 + HSH]], axis=3)
        vb = vb.transpose(1, 0, 2, 3, 4, 5).reshape(HSH, 162, HT)[:, :W, :]
        # [m,g,bank*12+j,n] -> [m, q=bank*48+g*12+j, n] -> [m,h,i,n]
        hb = np.asarray(results[core]["hband"]).astype(np.float32)
        hb = hb.reshape(MH, len(HBASES), NHBANK, HSLOT, NH)
        hb = hb.transpose(0, 2, 1, 3, 4).reshape(MH, NQ, NH)
        hb = hb.reshape(MH, HSH, NCH, NH)
        for s, (dh, dw) in enumerate(SHIFTS):
            if dw == 0:
                out[b, s, h0:h0 + HSH, :] = vb[
                    hidx[:, None], widx[None, :], (hidx + dh + R)[:, None]
                ]
            else:
                v = hb[
                    midx[:, None, None],
                    hidx[None, :, None],
                    iidx[None, None, :],
                    (midx + dw + R)[:, None, None],
                ]  # [m, h, i]
                out[b, s, h0:h0 + HSH, :] = v.transpose(1, 2, 0).reshape(HSH, W)
    return out


def kernel(src, tgt, **run_kwargs):
    nc = _get_nc()
    in_maps = shard_inputs(src, tgt)
    res = bass_utils.run_bass_kernel_spmd(
        nc, in_maps, core_ids=list(range(NCORES)), **run_kwargs
    )
    out = extract_output(res.results)
    kernel.last_result = res
    return out
